# revision 1
# baseline (speedup 1.0000x reference)
"""MixProp GNN message passing on 8 Trainium2 NeuronCores.

Reference computation (per batch element b):
    h0 = x;  h_k = alpha*x + (1-alpha) * (adj @ h_{k-1})   k=1..3   (matmul over nodes)
    ho = concat([h0..h3], channel axis);  out = W @ ho + b          (1x1 conv)

Node-propagation (node axis) commutes with channel mixing (channel
axis), so the alpha-blending folds into the conv weights on the host:
    out = sum_k M_k @ (A^k x) + b
with M_0 = W0 + a(W1+W2+W3), M_1 = B(W1 + aW2 + aW3),
     M_2 = B^2(W2 + aW3),    M_3 = B^3 W3,   (a=alpha, B=1-alpha)
leaving the device 3 chained propagation matmuls plus one K=128
channel-mix matmul.

Sharding: data-parallel over batch B=8, one batch element per core;
adj (host-pre-transposed) and conv weights replicated.

Device dataflow per core (fp16 operands, fp32 PSUM accumulation):
  X   [128 nodepart, 4 nodetile, 32c*168t]  <- DMA from host-cast x16[b]
  Y1 = A X ; Y2 = A Y1 ; Y3 = A Y2          (PE, contract node dim)
  each Y_k also lands in HBM scratch in TRANSPOSED fp16 layout [c,v,t]
  conv: re-read [32c part, (v,t)] slices of {x16, y1T, y2T, y3T}
  stacked on 128 partitions; groups of 4 column-tiled K=128 matmuls
  fill one [128, 512] PSUM tile concurrently; one DVE bias-add per
  group; DMA straight out via a strided scatter (free transpose).
"""

import sys

import numpy as np

sys.path.insert(0, "/opt/trn_rl_repo")

from contextlib import ExitStack

GDEP = 3
ALPHA = 0.05
Y3_SCALE = 1.0 / 128.0   # keep |y3| inside fp16 range; folded into M3
C = 32            # channels
N = 512           # nodes
T = 168           # time steps
B = 8             # batch == n_cores
P = 128           # partitions
NVT = N // P      # 4 node tiles
CT = C * T        # 5376 free columns in propagation layout
KC = (GDEP + 1) * C   # 128 stacked channels for the conv
VT_COLS = P * T   # 21504 flat (v,t) columns per node tile

# propagation free-dim chunks for steps 1/2 (psum bank = 512 fp32)
PROP_CHUNKS = [(i * 512, 512) for i in range(10)] + [(5120, 256)]
# conv: 42 sub-chunks of 512 per node tile, in groups of 4 (col-tiled)
CONV_GROUPS = [(m, min(4, 42 - 4 * m)) for m in range((42 + 3) // 4)]

_NC_CACHE = {}


def _build_nc():
    import concourse.mybir as mybir
    import concourse.tile as tile
    from concourse import bacc

    f32 = mybir.dt.float32
    f16 = mybir.dt.float16

    nc = bacc.Bacc("TRN2", target_bir_lowering=False, debug=False, num_devices=B)

    xb16 = nc.dram_tensor("xb16", [C, N, T], f16, kind="ExternalInput").ap()
    xprop = nc.dram_tensor("xprop", [P, NVT, C, T], f16, kind="ExternalInput").ap()
    adjT16 = nc.dram_tensor("adjT16", [N, N], f16, kind="ExternalInput").ap()
    mt16 = nc.dram_tensor("mt16", [KC, C], f16, kind="ExternalInput").ap()
    bias128 = nc.dram_tensor("bias128", [P, 512], f32, kind="ExternalInput").ap()
    out = nc.dram_tensor("out", [C, N, T], f32, kind="ExternalOutput").ap()
    ykT = [nc.dram_tensor(f"y{k}T", [C, N, T], f16).ap() for k in (1, 2, 3)]

    with tile.TileContext(nc) as tc, ExitStack() as ctx:
        _emit(ctx, tc, nc, mybir, xb16, xprop, adjT16, mt16, bias128, out, ykT)

    nc.compile()
    return nc


def _emit(ctx, tc, nc, mybir, xb16, xprop, adjT16, mt16, bias128, out, ykT):
    f32 = mybir.dt.float32
    f16 = mybir.dt.float16

    const_pool = ctx.enter_context(tc.tile_pool(name="const", bufs=1))
    chain_pool = ctx.enter_context(tc.tile_pool(name="chain", bufs=2))
    stage_pool = ctx.enter_context(tc.tile_pool(name="stage", bufs=2))
    psum_pool = ctx.enter_context(tc.tile_pool(name="psum", bufs=6, space="PSUM"))
    ho_pool = ctx.enter_context(tc.tile_pool(name="ho", bufs=2))
    cpsum_pool = ctx.enter_context(tc.tile_pool(name="cpsum", bufs=2, space="PSUM"))
    ostage_pool = ctx.enter_context(tc.tile_pool(name="ostage", bufs=4))

    # ---- load x in propagation layout first (host pre-swizzled, one
    # fully-contiguous DMA) — it is the PE's longest-pole start dep, so
    # it leads the HWDGE FIFO ----------------------------------------
    X = chain_pool.tile([P, NVT, CT], f16, tag="chain")
    nc.sync.dma_start(
        X[:].rearrange("p wt j -> p (wt j)"),
        xprop.rearrange("p wt c t -> p (wt c t)"),
    )

    # ---- adjacency next (PE's other start dependency) --------------
    adj_sb = const_pool.tile([P, NVT, N], f16, tag="adj")
    nc.sync.dma_start(adj_sb[:], adjT16.rearrange("(wt wp) v -> wp wt v", wp=P))

    # transposed-write view of the HBM scratch: dims (vp, c, t) for one vt
    def ykT_wview(k, vt):
        return ykT[k].rearrange("c (vt vp) t -> vt vp c t", vp=P)[vt]

    # ---- propagation steps 1 and 2 (keep result in SBUF + HBM copy) --
    # conv-input prefetch plumbing: each ho row is issued the moment its
    # source exists (x16 rows immediately, y1T/y2T rows as the steps
    # produce them) so the serial DMA stream never starves the conv
    srcs = [xb16] + ykT
    ho_tiles = {}

    def alloc_ho(vt):
        ho_t = ho_pool.tile([KC, VT_COLS], f16, tag="ho")
        ho_tiles[vt] = ho_t

    def load_ho_row(vt, k):
        nc.sync.dma_start(
            ho_tiles[vt][k * C:(k + 1) * C, :].rearrange("p (v t) -> p v t", t=T),
            srcs[k][:, vt * P:(vt + 1) * P, :],
        )

    for vt in (0, 1):
        alloc_ho(vt)
        load_ho_row(vt, 0)

    # conv constants last in the startup FIFO (needed ~150us later)
    mt_sb = const_pool.tile([KC, C], f16, tag="mt")
    nc.sync.dma_start(mt_sb[:], mt16)
    bias_sb = const_pool.tile([P, 512], f32, tag="bias")
    nc.sync.dma_start(bias_sb[:], bias128)

    cur = X
    for k in range(2):
        nxt = chain_pool.tile([P, NVT, CT], f16, tag="chain")
        for vt in range(NVT):
            # transposed write of this node tile to HBM in channel
            # halves, each emitted as soon as the psum copies covering
            # its channel range are in the stream (fills DMA idle)
            nxt_ctv = nxt[:, vt, :].rearrange("p (c t) -> p c t", t=T)
            for ji, (j0, jn) in enumerate(PROP_CHUNKS):
                ps = psum_pool.tile([P, 512], f32, tag="ps")
                for wt in range(NVT):
                    nc.tensor.matmul(
                        ps[:, :jn],
                        adj_sb[:, wt, vt * P:(vt + 1) * P],
                        cur[:, wt, j0:j0 + jn],
                        start=(wt == 0),
                        stop=(wt == NVT - 1),
                    )
                nc.vector.tensor_copy(nxt[:, vt, j0:j0 + jn], ps[:, :jn])
                if ji == 5:   # chunks 0-5 cover flat cols 0-3072 > 16ch
                    nc.sync.dma_start(
                        ykT_wview(k, vt)[:, 0:C // 2, :],
                        nxt_ctv[:, 0:C // 2, :],
                    )
            nc.sync.dma_start(
                ykT_wview(k, vt)[:, C // 2:C, :],
                nxt_ctv[:, C // 2:C, :],
            )
            if vt < 2:
                load_ho_row(vt, k + 1)
        cur = nxt

    # ---- step 3 + conv, conv lagged one node tile behind -----------
    # PE executes its stream in order: emitting conv(vt) immediately
    # after step3(vt) head-of-line-blocks ready step3(vt+1) matmuls
    # whenever conv(vt) waits on its y3 round trip. Lag the conv by one
    # tile so each conv has a full step-3 tile of PE work as slack.
    def emit_step3(vt):
        st = stage_pool.tile([P, CT], f16, tag="st")
        for j0, jn in PROP_CHUNKS:
            ps = psum_pool.tile([P, 512], f32, tag="ps")
            for wt in range(NVT):
                nc.tensor.matmul(
                    ps[:, :jn],
                    adj_sb[:, wt, vt * P:(vt + 1) * P],
                    cur[:, wt, j0:j0 + jn],
                    start=(wt == 0),
                    stop=(wt == NVT - 1),
                )
            nc.vector.tensor_scalar_mul(st[:, j0:j0 + jn], ps[:, :jn], Y3_SCALE)
        st_ctv = st[:].rearrange("p (c t) -> p c t", t=T)
        for c0 in (0, C // 2):
            nc.sync.dma_start(
                ykT_wview(2, vt)[:, c0:c0 + C // 2, :],
                st_ctv[:, c0:c0 + C // 2, :],
            )
        load_ho_row(vt, 3)

    def emit_conv(vt):
        # conv: ho[(k,c), (v,t)] stacked for one whole node tile; 4
        # consecutive 512-wide sub-chunks matmul'd concurrently into one
        # [128,512] psum via tile_position col groups
        ho = ho_tiles[vt]
        for m, gn in CONV_GROUPS:
            cps = cpsum_pool.tile([P, 512], f32, tag="cps")
            for j in range(gn):
                a = (4 * m + j) * 512
                nc.tensor.matmul(
                    cps[32 * j:32 * (j + 1), :],
                    mt_sb[:],
                    ho[:, a:a + 512],
                    start=True,
                    stop=True,
                    tile_position=(0, 32 * j),
                )
            ot = ostage_pool.tile([P, 512], f32, tag="ot")
            # psum evacuation + bias: ScalarE while DVE is busy with the
            # step-3 copies (vt 0/1), DVE in the tail where it idles
            if vt < 2:
                nc.scalar.activation(
                    ot[:32 * gn, :],
                    cps[:32 * gn, :],
                    mybir.ActivationFunctionType.Identity,
                    bias=bias_sb[:32 * gn, 0:1],
                )
            else:
                nc.vector.tensor_add(
                    ot[:32 * gn, :], cps[:32 * gn, :], bias_sb[:32 * gn, :]
                )
            # scatter rows (j, o) back to out[o, v, t]: global 512-chunk
            # index q = vt*42 + 4m + j
            q0 = vt * 42 + 4 * m
            dst = out.rearrange("o v t -> o (v t)").rearrange(
                "o (q i) -> q o i", i=512
            )[q0:q0 + gn]
            nc.sync.dma_start(dst, ot[:32 * gn, :])
        if vt + 2 < NVT:
            alloc_ho(vt + 2)
            for kk in range(3):
                load_ho_row(vt + 2, kk)

    emit_step3(0)
    emit_step3(1)
    emit_conv(0)
    emit_step3(2)
    emit_conv(1)
    emit_step3(3)
    emit_conv(2)
    emit_conv(3)


def _get_nc():
    if "nc" not in _NC_CACHE:
        _NC_CACHE["nc"] = _build_nc()
    return _NC_CACHE["nc"]


def _host_prep(adj, W, b):
    """Host-side constant folding: transposed adj, mixed conv weights."""
    a, beta = ALPHA, 1.0 - ALPHA
    W = np.asarray(W, dtype=np.float32)
    W0, W1, W2, W3 = (W[:, i * C:(i + 1) * C] for i in range(4))
    M0 = W0 + a * (W1 + W2 + W3)
    M1 = beta * (W1 + a * W2 + a * W3)
    M2 = beta * beta * (W2 + a * W3)
    M3 = beta * beta * beta * W3 / Y3_SCALE
    mt16 = np.ascontiguousarray(
        np.concatenate([M0.T, M1.T, M2.T, M3.T], axis=0)
    ).astype(np.float16)  # [128, 32]: row (k*32+c), col o = M_k[o, c]
    bias128 = np.ascontiguousarray(
        np.tile(np.asarray(b, dtype=np.float32)[:, None], (4, 512))
    )  # [128, 512]: row (j*32+o) = b[o]
    adjT16 = np.ascontiguousarray(np.asarray(adj, dtype=np.float32).T).astype(
        np.float16
    )
    return adjT16, mt16, bias128


def make_in_maps(x, adj, W, b):
    adjT16, mt16, bias128 = _host_prep(adj, W, b)
    x16 = np.ascontiguousarray(np.asarray(x, dtype=np.float32).astype(np.float16))
    xprop = np.ascontiguousarray(
        x16.reshape(B, C, NVT, P, T).transpose(0, 3, 2, 1, 4)
    )
    return [
        {
            "xb16": x16[i],
            "xprop": xprop[i],
            "adjT16": adjT16,
            "mt16": mt16,
            "bias128": bias128,
        }
        for i in range(B)
    ]


def _get_runner():
    """Reusable jitted SPMD executor (safe to invoke repeatedly, unlike
    per-call run_bass_kernel_spmd under axon)."""
    if "runner" in _NC_CACHE:
        return _NC_CACHE["runner"]
    import jax
    from jax.sharding import Mesh, PartitionSpec
    try:
        from jax import shard_map
    except ImportError:
        from jax.experimental.shard_map import shard_map
    from concourse import bass2jax, mybir

    nc = _get_nc()
    bass2jax.install_neuronx_cc_hook()

    pname = nc.partition_id_tensor.name if nc.partition_id_tensor else None
    in_names, out_names, out_avals, zero_outs = [], [], [], []
    for alloc in nc.m.functions[0].allocations:
        if not isinstance(alloc, mybir.MemoryLocationSet):
            continue
        name = alloc.memorylocations[0].name
        if alloc.kind == "ExternalInput":
            if name != pname:
                in_names.append(name)
        elif alloc.kind == "ExternalOutput":
            out_names.append(name)
            shape = tuple(alloc.tensor_shape)
            dtype = mybir.dt.np(alloc.dtype)
            out_avals.append(jax.core.ShapedArray(shape, dtype))
            zero_outs.append(np.zeros(shape, dtype))
    n_params = len(in_names)
    in_names_all = list(in_names) + out_names
    if pname is not None:
        in_names_all.append(pname)

    def _body(*args):
        operands = list(args)
        if pname is not None:
            operands.append(bass2jax.partition_id_tensor())
        return tuple(
            bass2jax._bass_exec_p.bind(
                *operands,
                out_avals=tuple(out_avals),
                in_names=tuple(in_names_all),
                out_names=tuple(out_names),
                lowering_input_output_aliases=(),
                sim_require_finite=True,
                sim_require_nnan=True,
                nc=nc,
            )
        )

    devices = jax.devices()[:B]
    mesh = Mesh(np.asarray(devices), ("core",))
    fn = jax.jit(
        shard_map(
            _body,
            mesh=mesh,
            in_specs=(PartitionSpec("core"),) * (n_params + len(out_names)),
            out_specs=(PartitionSpec("core"),) * len(out_names),
            check_rep=False,
        ),
        keep_unused=True,
    )

    def run(in_maps):
        per_core = [[np.asarray(m[nm]) for nm in in_names] for m in in_maps]
        concat_in = [
            np.concatenate([per_core[c][i] for c in range(B)], axis=0)
            for i in range(n_params)
        ]
        concat_zero = [np.concatenate([z] * B, axis=0) for z in zero_outs]
        outs = fn(*concat_in, *concat_zero)
        oi = out_names.index("out")
        full = np.asarray(outs[oi])
        per_core_rows = out_avals[oi].shape[0]
        return full.reshape(B, per_core_rows, *out_avals[oi].shape[1:])

    _NC_CACHE["runner"] = run
    return run


def kernel(x, adj, W, b):
    in_maps = make_in_maps(x, adj, W, b)
    try:
        run = _get_runner()
        return run(in_maps)
    except Exception:
        from concourse.bass_utils import run_bass_kernel_spmd

        res = run_bass_kernel_spmd(_get_nc(), in_maps, list(range(B)))
        return np.stack([res.results[i]["out"] for i in range(B)], axis=0)



# revision 2
# speedup vs baseline: 1.2641x; 1.2641x over previous
"""MixProp GNN message passing on 8 Trainium2 NeuronCores.

Reference computation (per batch element b):
    h0 = x;  h_k = alpha*x + (1-alpha) * (adj @ h_{k-1})   k=1..3   (matmul over nodes)
    ho = concat([h0..h3], channel axis);  out = W @ ho + b          (1x1 conv)

Node-propagation (node axis) commutes with channel mixing (channel
axis), so the alpha-blending folds into the conv weights on the host:
    out = sum_k M_k @ (A^k x) + b
with M_0 = W0 + a(W1+W2+W3), M_1 = B(W1 + aW2 + aW3),
     M_2 = B^2(W2 + aW3),    M_3 = B^3 W3,   (a=alpha, B=1-alpha)

Sharding: data-parallel over batch B=8, one batch element per core;
adj (host-pre-transposed) and conv weights replicated.

Device dataflow per core (fp16 operands, fp32 PSUM accumulation),
pipelined over T in chunks of 16 time steps:
  YK [128 w-part, 4 node tiles, 16 t, 128 kc]   kc = k*32 + c
    slot k=0 <- x (ACT copy from the contiguous chunk load)
    slot k   <- prop step k psum (PE contracts nodes; DVE/ACT evac)
  per node tile: XBAR DMA-transpose [v, (t,kc)] -> Z [kc, t, v] on-chip
    (no HBM scratch round trip)
  conv: one K=128 matmul per (node tile, t): psum[128 v, 32 o] with
    ap_size=32 (4x fewer PE rows than the [o, (v,t)] orientation)
  psum -> stage [v, o, t] f32 (DVE add folds the bias, (t,o)->(o,t))
  out DMA per node tile in two slabs: t 0:128 (512B descriptors,
  overlapped with the tail chunks) + t 128:168.
"""

import sys

import numpy as np

sys.path.insert(0, "/opt/trn_rl_repo")

from contextlib import ExitStack

GDEP = 3
ALPHA = 0.05
Y3_SCALE = 1.0 / 128.0   # keep |y3| inside fp16 range; folded into M3
C = 32            # channels
N = 512           # nodes
T = 168           # time steps
B = 8             # batch == n_cores
P = 128           # partitions
NVT = N // P      # 4 node tiles
KC = (GDEP + 1) * C   # 128 stacked (k, c) rows for the conv

TC = 16                                      # t-chunk size
CHUNKS = [(i * TC, TC) for i in range(10)] + [(160, 8)]

_NC_CACHE = {}


def _build_nc():
    import concourse.mybir as mybir
    import concourse.tile as tile
    from concourse import bacc

    f32 = mybir.dt.float32
    f16 = mybir.dt.float16

    nc = bacc.Bacc("TRN2", target_bir_lowering=False, debug=False, num_devices=B)

    xprop = nc.dram_tensor("xprop", [P, NVT, T, C], f16, kind="ExternalInput").ap()
    adjT16 = nc.dram_tensor("adjT16", [N, N], f16, kind="ExternalInput").ap()
    mt16 = nc.dram_tensor("mt16", [KC, C], f16, kind="ExternalInput").ap()
    bias512 = nc.dram_tensor("bias512", [P, 512], f32, kind="ExternalInput").ap()
    out = nc.dram_tensor("out", [C, N, T], f32, kind="ExternalOutput").ap()

    with tile.TileContext(nc) as tc, ExitStack() as ctx:
        _emit(ctx, tc, nc, mybir, xprop, adjT16, mt16, bias512, out)

    nc.compile()
    return nc


def _emit(ctx, tc, nc, mybir, xprop, adjT16, mt16, bias512, out):
    f32 = mybir.dt.float32
    f16 = mybir.dt.float16
    Identity = mybir.ActivationFunctionType.Identity

    const_pool = ctx.enter_context(tc.tile_pool(name="const", bufs=1))
    xp_pool = ctx.enter_context(tc.tile_pool(name="xp", bufs=2))
    yk_pool = ctx.enter_context(tc.tile_pool(name="yk", bufs=2))
    z_pool = ctx.enter_context(tc.tile_pool(name="z", bufs=6))
    psum_pool = ctx.enter_context(tc.tile_pool(name="psum", bufs=3, space="PSUM"))
    cpsum_pool = ctx.enter_context(tc.tile_pool(name="cpsum", bufs=3, space="PSUM"))

    # ---- adjacency first (PE's start dependency) -------------------
    adj_sb = const_pool.tile([P, NVT, N], f16, tag="adj")
    nc.sync.dma_start(adj_sb[:], adjT16.rearrange("(wt wp) v -> wp wt v", wp=P))

    # output staging: [v, vt, o, t] f32, written (o,t)-transposed per
    # conv psum, flushed to HBM in two t-slabs per node tile
    stage = const_pool.tile([P, NVT, C, T], f32, tag="stage")

    # conv constants (needed ~one chunk in)
    mt_sb = const_pool.tile([KC, C], f16, tag="mt")
    bias_sb = const_pool.tile([P, 512], f32, tag="bias")

    consts_loaded = [False]

    def load_consts():
        nc.sync.dma_start(mt_sb[:], mt16)
        nc.sync.dma_start(bias_sb[:], bias512)
        consts_loaded[0] = True

    def emit_chunk(t0, tn):
        """Load x chunk, run the 3 propagation steps, XBAR-transpose
        each node tile; returns state for the (lagged) conv."""
        xp = xp_pool.tile([P, NVT, TC, C], f16, tag="xp")
        nc.sync.dma_start(xp[:, :, :tn, :], xprop[:, :, t0:t0 + tn, :])
        yk = yk_pool.tile([P, NVT, TC, KC], f16, tag="yk")
        # x -> stacked slot k=0 (ACT; strided dst)
        nc.scalar.activation(yk[:, :, :tn, 0:C], xp[:, :, :tn, :], Identity)
        zs = []
        for k in (1, 2, 3):
            for vt in range(NVT):
                ps = psum_pool.tile([P, 512], f32, tag="ps")
                for wt in range(NVT):
                    rhs = (xp[:, wt, :tn, :] if k == 1
                           else yk[:, wt, :tn, C * (k - 1):C * k])
                    nc.tensor.matmul(
                        ps[:, :tn * C],
                        adj_sb[:, wt, vt * P:(vt + 1) * P],
                        rhs,
                        start=(wt == 0),
                        stop=(wt == NVT - 1),
                    )
                src = ps[:, :tn * C].rearrange("p (t c) -> p t c", c=C)
                dst = yk[:, vt, :tn, C * k:C * (k + 1)]
                if k == 3:
                    # scale guards fp16 range; ACT while DVE drains 1/2
                    nc.scalar.activation(dst, src, Identity, scale=Y3_SCALE)
                    z = z_pool.tile([P, TC, P], f16, tag="z")
                    nc.sync.dma_start(
                        z[:, :tn, :], yk[:, vt, :tn, :], transpose=True
                    )
                    zs.append(z)
                else:
                    nc.vector.tensor_copy(dst, src)
        return (t0, tn, zs)

    def emit_conv(state):
        t0, tn, zs = state
        for vt in range(NVT):
            z = zs[vt]
            cps = cpsum_pool.tile([P, 512], f32, tag="cps")
            for i in range(tn):
                nc.tensor.matmul(
                    cps[:, C * i:C * (i + 1)],
                    z[:, i, :],
                    mt_sb[:],
                    start=True,
                    stop=True,
                )
            # evac + bias, (t,o) -> (o,t) reorder into the stage
            nc.vector.tensor_add(
                stage[:, vt, :, t0:t0 + tn].rearrange("p o t -> p t o"),
                cps[:, :tn * C].rearrange("p (t o) -> p t o", o=C),
                bias_sb[:, :tn * C].rearrange("p (t o) -> p t o", o=C),
            )

    out_v = out.rearrange("o (vt v) t -> vt v o t", v=P)

    def flush(vt, ta, tb):
        nc.sync.dma_start(out_v[vt][:, :, ta:tb], stage[:, vt, :, ta:tb])

    pending = None
    for ci, (t0, tn) in enumerate(CHUNKS):
        state = emit_chunk(t0, tn)
        if ci == 0:
            load_consts()
        if pending is not None:
            emit_conv(pending)
            pt0, ptn, _ = pending
            if pt0 + ptn == 128:     # t 0:128 done -> big-descriptor slab
                for vt in range(NVT):
                    flush(vt, 0, 128)
        pending = state
    emit_conv(pending)
    for vt in range(NVT):
        flush(vt, 128, T)


def _get_nc():
    if "nc" not in _NC_CACHE:
        _NC_CACHE["nc"] = _build_nc()
    return _NC_CACHE["nc"]


def _host_prep(adj, W, b):
    """Host-side constant folding: transposed adj, mixed conv weights."""
    a, beta = ALPHA, 1.0 - ALPHA
    W = np.asarray(W, dtype=np.float32)
    W0, W1, W2, W3 = (W[:, i * C:(i + 1) * C] for i in range(4))
    M0 = W0 + a * (W1 + W2 + W3)
    M1 = beta * (W1 + a * W2 + a * W3)
    M2 = beta * beta * (W2 + a * W3)
    M3 = beta * beta * beta * W3 / Y3_SCALE
    mt16 = np.ascontiguousarray(
        np.concatenate([M0.T, M1.T, M2.T, M3.T], axis=0)
    ).astype(np.float16)  # [128, 32]: row (k*32+c), col o = M_k[o, c]
    bias512 = np.ascontiguousarray(
        np.tile(np.asarray(b, dtype=np.float32)[None, :], (P, TC))
    )  # [128, 512]: col (t'*32+o) = b[o]
    adjT16 = np.ascontiguousarray(np.asarray(adj, dtype=np.float32).T).astype(
        np.float16
    )
    return adjT16, mt16, bias512


def make_in_maps(x, adj, W, b):
    adjT16, mt16, bias512 = _host_prep(adj, W, b)
    x16 = np.asarray(x, dtype=np.float32).astype(np.float16)
    # [B, C, N, T] -> [B, 128 wp, 4 wt, T, C]
    xprop = np.ascontiguousarray(
        x16.reshape(B, C, NVT, P, T).transpose(0, 3, 2, 4, 1)
    )
    return [
        {
            "xprop": xprop[i],
            "adjT16": adjT16,
            "mt16": mt16,
            "bias512": bias512,
        }
        for i in range(B)
    ]


def _get_runner():
    """Reusable jitted SPMD executor (safe to invoke repeatedly, unlike
    per-call run_bass_kernel_spmd under axon)."""
    if "runner" in _NC_CACHE:
        return _NC_CACHE["runner"]
    import jax
    from jax.sharding import Mesh, PartitionSpec
    try:
        from jax import shard_map
    except ImportError:
        from jax.experimental.shard_map import shard_map
    from concourse import bass2jax, mybir

    nc = _get_nc()
    bass2jax.install_neuronx_cc_hook()

    pname = nc.partition_id_tensor.name if nc.partition_id_tensor else None
    in_names, out_names, out_avals, zero_outs = [], [], [], []
    for alloc in nc.m.functions[0].allocations:
        if not isinstance(alloc, mybir.MemoryLocationSet):
            continue
        name = alloc.memorylocations[0].name
        if alloc.kind == "ExternalInput":
            if name != pname:
                in_names.append(name)
        elif alloc.kind == "ExternalOutput":
            out_names.append(name)
            shape = tuple(alloc.tensor_shape)
            dtype = mybir.dt.np(alloc.dtype)
            out_avals.append(jax.core.ShapedArray(shape, dtype))
            zero_outs.append(np.zeros(shape, dtype))
    n_params = len(in_names)
    in_names_all = list(in_names) + out_names
    if pname is not None:
        in_names_all.append(pname)

    def _body(*args):
        operands = list(args)
        if pname is not None:
            operands.append(bass2jax.partition_id_tensor())
        return tuple(
            bass2jax._bass_exec_p.bind(
                *operands,
                out_avals=tuple(out_avals),
                in_names=tuple(in_names_all),
                out_names=tuple(out_names),
                lowering_input_output_aliases=(),
                sim_require_finite=True,
                sim_require_nnan=True,
                nc=nc,
            )
        )

    devices = jax.devices()[:B]
    mesh = Mesh(np.asarray(devices), ("core",))
    fn = jax.jit(
        shard_map(
            _body,
            mesh=mesh,
            in_specs=(PartitionSpec("core"),) * (n_params + len(out_names)),
            out_specs=(PartitionSpec("core"),) * len(out_names),
            check_rep=False,
        ),
        keep_unused=True,
    )

    def run(in_maps):
        per_core = [[np.asarray(m[nm]) for nm in in_names] for m in in_maps]
        concat_in = [
            np.concatenate([per_core[c][i] for c in range(B)], axis=0)
            for i in range(n_params)
        ]
        concat_zero = [np.concatenate([z] * B, axis=0) for z in zero_outs]
        outs = fn(*concat_in, *concat_zero)
        oi = out_names.index("out")
        full = np.asarray(outs[oi])
        per_core_rows = out_avals[oi].shape[0]
        return full.reshape(B, per_core_rows, *out_avals[oi].shape[1:])

    _NC_CACHE["runner"] = run
    return run


def kernel(x, adj, W, b):
    in_maps = make_in_maps(x, adj, W, b)
    try:
        run = _get_runner()
        return run(in_maps)
    except Exception:
        from concourse.bass_utils import run_bass_kernel_spmd

        res = run_bass_kernel_spmd(_get_nc(), in_maps, list(range(B)))
        return np.stack([res.results[i]["out"] for i in range(B)], axis=0)
